# revision 1
# baseline (speedup 1.0000x reference)
"""Multi-head self-attention (N=2, S=4096, D=1024, H=16) on 8 trn2 cores.

Sharding: data-parallel over batch (2) x tensor-parallel over head groups
(4 heads per core). Core c handles batch b=c//4, head group g=c%4
(heads 4g..4g+3, i.e. output columns 256g..256g+256). No cross-device
comms: heads are independent.

Per-core device kernel:
  - Projections in fp16 (full PE rate, FWL weight loads, half DMA):
    qT [256,4096], per-head kTz planes [128,4096] (off-parity rows
    zeroed so the S matmul runs K=128: a half-idle PE array reads as
    "inactive" to the HAM clock gate and gets throttled to half clock),
    and v in bf16 ("vaug": 64 v cols + a ones column per head, padded
    so the PV weight load is always 128 wide -> full array + FWL; the
    extra output rows land in PSUM pad space and row 64 accumulates
    the softmax denominator for free).
  - Attention per head, flash-style: ST chunk [j=128, i=1024] on PE
    (fp16 in, fp32 accumulate), exp on ScalarE straight from PSUM
    (bf16 out), PV accumulation on PE.
  - Epilogue per 128-query block: PE-transpose, DVE reciprocal+scale,
    DMA out.
  - Emission is interleaved: the k/q projections for the first query
    i-chunk run first, then v/q projections are woven between early
    attention units so ScalarE starts exp'ing ~40us in instead of
    ~120us. PSUM (8 banks): ST double-buffer 2x2 + OT 2 + projection/
    epilogue pool 2x1.
"""

import numpy as np

import concourse.bacc as bacc
import concourse.tile as tile
import concourse.mybir as mybir
from concourse.bass_utils import run_bass_kernel_spmd
from concourse.masks import make_identity

F32 = mybir.dt.float32
F32R = mybir.dt.float32r
BF16 = mybir.dt.bfloat16
FP16 = mybir.dt.float16
Exp = mybir.ActivationFunctionType.Exp

N, S, D = 2, 4096, 1024
H = 16
HD = D // H                      # 64
N_CORES = 8
HPC = H // (N_CORES // N)        # heads per core = 4
MPC = HPC * HD                   # out columns per core = 256
SCALE = 1.0 / np.sqrt(HD)        # post-matmul softmax scale

IC = 1024                        # i-chunk (query cols per exp instruction)
N_IC = S // IC                   # 4
N_JC = S // 128                  # 32 key chunks
N_SC = S // 512                  # 8 projection s-chunks
N_DT = D // 128                  # 8 contraction tiles
VW = HD + 1                      # vaug stride per head (64 v + 1 ones)


def build_attention_kernel():
    nc = bacc.Bacc(
        "TRN2", target_bir_lowering=False, debug=False,
        enable_asserts=False, num_devices=N_CORES,
    )
    xT = nc.dram_tensor("xT", [D, S], FP16, kind="ExternalInput").ap()
    wqT = nc.dram_tensor("wqT", [D, MPC], FP16, kind="ExternalInput").ap()
    wkT = nc.dram_tensor("wkT", [D, MPC], FP16, kind="ExternalInput").ap()
    wvT = nc.dram_tensor("wvT", [D, MPC], FP16, kind="ExternalInput").ap()
    out = nc.dram_tensor("out", [S, MPC], F32, kind="ExternalOutput").ap()

    with tile.TileContext(nc) as tc:
        _emit(tc, xT, wqT, wkT, wvT, out)
    nc.compile()
    return nc


def _emit(tc, xT, wqT, wkT, wvT, out):
    nc = tc.nc
    with (
        tc.tile_pool(name="persist", bufs=1) as persist,
        # PSUM (8 banks): st 2x2 + ot 1x2 + proj 2x1 = 8; epilogue
        # transposes borrow "st" slots (FIFO order puts them right
        # after the unit's last ST chunk).
        tc.tile_pool(name="stp", bufs=2, space="PSUM") as stp,
        tc.tile_pool(name="otp", bufs=1, space="PSUM") as otp,
        tc.tile_pool(name="prp", bufs=2, space="PSUM") as prp,
        tc.tile_pool(name="xload", bufs=2) as xload,
        tc.tile_pool(name="esb", bufs=3) as esb,
        tc.tile_pool(name="episb", bufs=2) as episb,
        tc.tile_pool(name="osb", bufs=3) as osb,
    ):
        w_sb = {}
        for name, w in (("q", wqT), ("k", wkT), ("v", wvT)):
            t = persist.tile([128, N_DT, MPC], FP16, tag=f"w{name}")
            for dt in range(N_DT):
                nc.sync.dma_start(out=t[:, dt, :], in_=w[dt * 128:(dt + 1) * 128, :])
            w_sb[name] = t
        qT_sb = persist.tile([128, 2, S], FP16, tag="qT")   # [m 2x128, s]
        kTz = persist.tile([128, HPC, S], FP16, tag="kTz")
        vaug = persist.tile([128, N_JC, HPC * VW + HD - 1], BF16, tag="vaug")
        ident = persist.tile([128, 128], F32, tag="ident")
        make_identity(nc, ident)

        ones_src = persist.tile([128, HPC], F32, tag="ones")
        nc.vector.memset(ones_src, 1.0)
        zero_src = persist.tile([128, 512], F32, tag="zeros")
        nc.vector.memset(zero_src, 0.0)
        for h in range(HPC):            # zero off-parity kTz rows
            z0 = 64 if h % 2 == 0 else 0
            for sc in range(N_SC):
                nc.vector.tensor_copy(
                    kTz[z0:z0 + 64, h, sc * 512:(sc + 1) * 512],
                    zero_src[z0:z0 + 64, :],
                )
        for jc in range(N_JC):          # vaug ones + zero pad columns
            nc.vector.tensor_copy(
                vaug[:, jc, 0:HPC * VW].rearrange(
                    "p (h c) -> p h c", c=VW)[:, :, HD:HD + 1],
                ones_src[:].rearrange("p (h c) -> p h c", c=1),
            )
            nc.vector.memset(vaug[:, jc, HPC * VW:], 0.0)

        # ---------- projection helpers ----------
        def load_x(sc):
            s0 = sc * 512
            x_t = xload.tile([128, N_DT, 512], FP16, tag="x")
            for dt in range(N_DT):
                nc.sync.dma_start(
                    out=x_t[:, dt, :],
                    in_=xT[dt * 128:(dt + 1) * 128, s0:s0 + 512],
                )
            return x_t

        def proj_qk(sc, x_t, name):
            s0 = sc * 512
            for mt in range(2):
                ps = prp.tile([128, 512], F32, tag="pr")
                for dt in range(N_DT):
                    nc.tensor.matmul(
                        ps[:],
                        w_sb[name][:, dt, mt * 128:(mt + 1) * 128],
                        x_t[:, dt, :],
                        start=(dt == 0), stop=(dt == N_DT - 1),
                    )
                if name == "q":
                    nc.vector.tensor_copy(qT_sb[:, mt, s0:s0 + 512], ps[:])
                else:
                    for hh in range(2):
                        p0 = hh * HD
                        nc.vector.tensor_copy(
                            kTz[p0:p0 + HD, mt * 2 + hh, s0:s0 + 512],
                            ps[p0:p0 + HD, :],
                        )

        def proj_v(sc, x_t):
            for st in range(4):
                ps = prp.tile([128, MPC], F32, tag="pr")
                for dt in range(N_DT):
                    nc.tensor.matmul(
                        ps[:],
                        x_t[:, dt, st * 128:(st + 1) * 128],
                        w_sb["v"][:, dt, :],
                        start=(dt == 0), stop=(dt == N_DT - 1),
                    )
                jc = sc * 4 + st
                nc.vector.tensor_copy(
                    vaug[:, jc, 0:HPC * VW].rearrange(
                        "p (h c) -> p h c", c=VW)[:, :, 0:HD],
                    ps[:].rearrange("p (h d) -> p h d", d=HD),
                )

        # ---------- attention helpers ----------
        def attn_unit(h, ic, jc, ot_ps):
            mt = h // 2
            i0 = ic * IC
            j0 = jc * 128
            st_ps = stp.tile([128, IC], F32, tag="st")
            for half in range(IC // 512):
                c0 = half * 512
                nc.tensor.matmul(
                    st_ps[:, c0:c0 + 512],
                    kTz[:, h, j0:j0 + 128],
                    qT_sb[:, mt, i0 + c0:i0 + c0 + 512],
                    start=True, stop=True,
                )
            e_t = esb.tile([128, IC], BF16, tag="e")
            nc.scalar.activation(e_t[:], st_ps[:], Exp, bias=0.0, scale=SCALE)
            lhsT_v = vaug[:, jc, h * VW:h * VW + 128]
            for half in range(IC // 512):
                c0 = half * 512
                nc.tensor.matmul(
                    ot_ps[:, c0:c0 + 512],
                    lhsT_v,
                    e_t[:, c0:c0 + 512],
                    start=(jc == 0), stop=(jc == N_JC - 1),
                )

        def epilogue(h, ic, ot_ps):
            i0 = ic * IC
            ot_sb = episb.tile([HD + 1, IC], F32, tag="eo")
            nc.vector.tensor_copy(ot_sb[:], ot_ps[0:HD + 1, :])
            for bi in range(IC // 128):
                tr = prp.tile([128, HD + 1], F32, tag="pr")
                nc.tensor.transpose(
                    tr[:],
                    ot_sb[:, bi * 128:(bi + 1) * 128],
                    ident[0:HD + 1, 0:HD + 1],
                )
                rec = osb.tile([128, 1], F32, tag="rec")
                nc.vector.reciprocal(rec[:], tr[:, HD:HD + 1])
                o_t = osb.tile([128, HD], F32, tag="o")
                nc.vector.tensor_scalar_mul(o_t[:], tr[:, 0:HD], rec[:])
                r0 = i0 + bi * 128
                nc.sync.dma_start(
                    out=out[r0:r0 + 128, h * HD:(h + 1) * HD],
                    in_=o_t[:],
                )

        # ---------- interleaved schedule ----------
        # k projections (+ q for s-chunks 0,1 = query i-chunk 0)
        for sc in range(N_SC):
            x_t = load_x(sc)
            proj_qk(sc, x_t, "k")
            if sc < 2:
                proj_qk(sc, x_t, "q")
        # head 0, i-chunk 0: v projections (bf16) woven in; q s-chunks
        # 2,3 ride the same loop so i-chunk 1 is ready next
        ot_ps = otp.tile([128, IC], F32, tag="ot")
        for sc in range(N_SC):
            x_t = load_x(sc)
            proj_v(sc, x_t)
            if sc in (2, 3):
                proj_qk(sc, x_t, "q")
            for jc in range(sc * 4, sc * 4 + 4):
                attn_unit(0, 0, jc, ot_ps)
        epilogue(0, 0, ot_ps)
        # head 0, i-chunk 1: remaining q projections woven in
        ot_ps = otp.tile([128, IC], F32, tag="ot")
        for jc in range(N_JC):
            if jc % 8 == 0:
                sc = 4 + jc // 8
                x_t = load_x(sc)
                proj_qk(sc, x_t, "q")
            attn_unit(0, 1, jc, ot_ps)
        epilogue(0, 1, ot_ps)
        # the rest: pure attention
        for h in range(HPC):
            for ic in range(N_IC):
                if h == 0 and ic < 2:
                    continue
                ot_ps = otp.tile([128, IC], F32, tag="ot")
                for jc in range(N_JC):
                    attn_unit(h, ic, jc, ot_ps)
                epilogue(h, ic, ot_ps)


_NC_CACHE = None


def _get_nc():
    global _NC_CACHE
    if _NC_CACHE is None:
        _NC_CACHE = build_attention_kernel()
    return _NC_CACHE


def _build_in_maps(inputs):
    x = np.asarray(inputs["x"], dtype=np.float32)
    Wq = np.asarray(inputs["Wq"], dtype=np.float32)
    Wk = np.asarray(inputs["Wk"], dtype=np.float32)
    Wv = np.asarray(inputs["Wv"], dtype=np.float32)
    xTs = [np.ascontiguousarray(x[b].T).astype(np.float16)
           for b in range(N)]
    in_maps = []
    for c in range(N_CORES):
        b, g = divmod(c, N_CORES // N)
        rows = slice(g * MPC, (g + 1) * MPC)
        in_maps.append({
            "xT": xTs[b],
            "wqT": np.ascontiguousarray(Wq[rows].T).astype(np.float16),
            "wkT": np.ascontiguousarray(Wk[rows].T).astype(np.float16),
            "wvT": np.ascontiguousarray(Wv[rows].T).astype(np.float16),
        })
    return in_maps


def kernel(x, Wq, Wk, Wv):
    nc = _get_nc()
    in_maps = _build_in_maps({"x": x, "Wq": Wq, "Wk": Wk, "Wv": Wv})
    res = run_bass_kernel_spmd(nc, in_maps, core_ids=list(range(N_CORES)))

    full = np.empty((N, S, D), dtype=np.float32)
    for c in range(N_CORES):
        b, g = divmod(c, N_CORES // N)
        full[b, :, g * MPC:(g + 1) * MPC] = res.results[c]["out"]
    return full


if __name__ == "__main__":
    rng = np.random.default_rng(0)
    x = rng.standard_normal((N, S, D)).astype(np.float32)
    Wq = (rng.standard_normal((D, D)) / 32).astype(np.float32)
    Wk = (rng.standard_normal((D, D)) / 32).astype(np.float32)
    Wv = (rng.standard_normal((D, D)) / 32).astype(np.float32)
    got = kernel(x, Wq, Wk, Wv)
    print("kernel output:", got.shape, got.dtype)

